# revision 6
# baseline (speedup 1.0000x reference)
"""Multi-head attention forward on 8 TRN2 NeuronCores.

Problem: x[2,2048,1024] @ {Wq,Wk,Wv}[1024,1024] (+bias) -> 16 heads of 64,
softmax(QK^T/8)V per head, concat -> @Wo[1024,1024] + bo.

Sharding: tensor-parallel over d_hid. Core c owns 2 heads (128 dims):
  - computes Q^T,K^T,V^T slices [128, 2048] per batch from full x^T
  - attention for its (2 batches x 2 heads)
  - partial out = ctx_slice @ Wo[slice_rows] -> [4096, 1024] (f16)
Host sums the 8 partials and adds bo (pure reduction, no collectives).

Key layout/perf decisions:
  - x^T [1024, 4096] uploaded pre-transposed, f16, loaded ONCE for both
    batches (8MB SBUF) so batch-1 projections never wait on DMA.
  - Wq, bq pre-scaled by 1/8 on host (folds the softmax scale).
  - No fp32/fp32r matmuls anywhere (they trip the HAM 0.5x util throttle):
    projections/scores f16, V-transposes f16, out-projection f16.
  - Scores computed transposed (S^T[k,q]); softmax normalization comes
    from a ones-column augmented V (row 64 of the ctx psum = row sums).
  - exp(scores) written as fp8e4 and V_aug stored fp8e4 in k-tile-pair
    interleaved layout [128, 2, 130] so the ctx accumulation runs in
    MatmulPerfMode.DoubleRow (2 k-tiles per pass, 0.5 cycles/row).
  - out partials staged via gpsimd (DVE stays on bias/norm work) and
    DMA'd out as f16 (halves write traffic).
  - PSUM banks: scores h0/h1 (2+2) + ctx h0/h1 (1+1) + proj/transpose/
    outproj pool (2) = 8.
"""

import os
import numpy as np

B, S, D = 2, 2048, 1024
NCORES = 8
HSLICE = D // NCORES          # 128 = 2 heads x 64
KT_PROJ = D // 128            # 8 contraction tiles for projections
NKT = S // 128                # 16 k-tiles per batch for attention
NKP = NKT // 2                # 8 k-tile pairs
QH = 512                      # q chunk (1 PSUM bank)
CH = 512                      # matmul free-dim chunk
BS = B * S

FP8_CTX = False               # fp8e4 exp/V + DoubleRow ctx matmul
                              # (disabled: ~1.8e-2 rel err, over budget)

_cache = {}


def _build():
    import concourse.bacc as bacc
    import concourse.tile as tile
    from concourse import mybir

    f32 = mybir.dt.float32
    f16 = mybir.dt.float16
    f8 = mybir.dt.float8e4
    AF = mybir.ActivationFunctionType
    MPM = mybir.MatmulPerfMode

    et_dt = f8 if FP8_CTX else f16

    nc = bacc.Bacc("TRN2", target_bir_lowering=False, debug=False,
                   num_devices=NCORES)

    xt_d = nc.dram_tensor("xt", [D, BS], f16, kind="ExternalInput").ap()
    wq_d = nc.dram_tensor("wq", [D, HSLICE], f16, kind="ExternalInput").ap()
    wk_d = nc.dram_tensor("wk", [D, HSLICE], f16, kind="ExternalInput").ap()
    wv_d = nc.dram_tensor("wv", [D, HSLICE], f16, kind="ExternalInput").ap()
    bq_d = nc.dram_tensor("bq", [HSLICE, 1], f32, kind="ExternalInput").ap()
    bk_d = nc.dram_tensor("bk", [HSLICE, 1], f32, kind="ExternalInput").ap()
    bv_d = nc.dram_tensor("bv", [HSLICE, 1], f32, kind="ExternalInput").ap()
    wo_d = nc.dram_tensor("wo", [HSLICE, D], f16, kind="ExternalInput").ap()
    idt_d = nc.dram_tensor("idt", [128, 128], f16, kind="ExternalInput").ap()
    out_d = nc.dram_tensor("out", [BS, D], f16, kind="ExternalOutput").ap()

    with tile.TileContext(nc) as tc:
        with (
            tc.tile_pool(name="wpool", bufs=1) as wpool,
            tc.tile_pool(name="xt", bufs=1) as xtp,
            tc.tile_pool(name="qk", bufs=2) as qkp,
            tc.tile_pool(name="vtmp", bufs=2) as vtp,
            tc.tile_pool(name="vaug", bufs=2) as vap,
            tc.tile_pool(name="et", bufs=2) as etp,
            tc.tile_pool(name="ctx", bufs=2) as ctxp,
            tc.tile_pool(name="norm", bufs=2) as normp,
            tc.tile_pool(name="ost", bufs=3) as ostp,
            tc.tile_pool(name="psS", bufs=1, space="PSUM") as psS,
            tc.tile_pool(name="psC", bufs=1, space="PSUM") as psC,
            tc.tile_pool(name="psP", bufs=2, space="PSUM") as psP,
        ):
            # ---- constants / weights ----
            wq_t, wk_t, wv_t = [], [], []
            for ki in range(KT_PROJ):
                for lst, src, tag in ((wq_t, wq_d, "wq"), (wk_t, wk_d, "wk"),
                                      (wv_t, wv_d, "wv")):
                    t = wpool.tile([128, HSLICE], f16, tag=f"{tag}{ki}")
                    nc.scalar.dma_start(t[:], src[ki * 128:(ki + 1) * 128, :])
                    lst.append(t)
            wo_t = wpool.tile([128, D], f16, tag="wo")
            nc.scalar.dma_start(wo_t[:], wo_d[:])
            idt = wpool.tile([128, 128], f16, tag="idt")
            nc.scalar.dma_start(idt[:], idt_d[:])
            bq_t = wpool.tile([128, 1], f32, tag="bq")
            nc.scalar.dma_start(bq_t[:], bq_d[:])
            bk_t = wpool.tile([128, 1], f32, tag="bk")
            nc.scalar.dma_start(bk_t[:], bk_d[:])
            bv_t = wpool.tile([128, 1], f32, tag="bv")
            nc.scalar.dma_start(bv_t[:], bv_d[:])

            # ---- load x^T for BOTH batches, column-chunk-major so proj
            # chunk c can start once its strip has landed ----
            xts = []
            for ki in range(KT_PROJ):
                t = xtp.tile([128, BS], f16, tag=f"xt{ki}")
                xts.append(t)
            XC = 1024
            for c in range(BS // XC):
                for ki in range(KT_PROJ):
                    nc.sync.dma_start(
                        xts[ki][:, c * XC:(c + 1) * XC],
                        xt_d[ki * 128:(ki + 1) * 128, c * XC:(c + 1) * XC])

            for b in range(B):
                s0 = b * S
                # ---- projections: Q^T (split per head, zero-padded), K^T,
                # V^T [128, 2048]. qth[h] has the other head's 64 rows zeroed
                # so scores can run full-K=128 matmuls.
                qt0 = qkp.tile([128, S], f16, tag="qt0")
                qt1 = qkp.tile([128, S], f16, tag="qt1")
                qth = [qt0, qt1]
                nc.vector.memset(qt0[64:128, :], 0.0)
                nc.vector.memset(qt1[0:64, :], 0.0)
                kt = qkp.tile([128, S], f16, tag="kt")
                vt = vtp.tile([128, S], f16, tag="vt")
                for di, (dst, w_t, b_t) in enumerate(
                        ((None, wq_t, bq_t), (kt, wk_t, bk_t), (vt, wv_t, bv_t))):
                    for c in range(S // CH):
                        ps = psP.tile([128, CH], f32, tag="pp")
                        for ki in range(KT_PROJ):
                            nc.tensor.matmul(ps[:], w_t[ki][:],
                                             xts[ki][:, s0 + c * CH:s0 + (c + 1) * CH],
                                             start=(ki == 0),
                                             stop=(ki == KT_PROJ - 1))
                        if dst is None:
                            nc.vector.tensor_scalar_add(
                                qt0[0:64, c * CH:(c + 1) * CH],
                                ps[0:64, :], b_t[0:64, 0:1])
                            nc.vector.tensor_scalar_add(
                                qt1[64:128, c * CH:(c + 1) * CH],
                                ps[64:128, :], b_t[64:128, 0:1])
                        else:
                            nc.vector.tensor_scalar_add(
                                dst[:, c * CH:(c + 1) * CH], ps[:], b_t[:, 0:1])

                # ---- V^T -> V_aug pair tiles [128, 2, 130] (ones at cols
                # 64, 129 of each subtile; head h dims at h*65..h*65+64) ----
                vaugs = []
                for kp in range(NKP):
                    va = vap.tile([128, 2, 130], et_dt, tag=f"va{kp}")
                    for j in range(2):
                        ki = 2 * kp + j
                        pst = psP.tile([128, 128], f16, tag="pp")
                        nc.tensor.transpose(pst[:],
                                            vt[:, ki * 128:(ki + 1) * 128],
                                            idt[:])
                        nc.vector.tensor_copy(va[:, j, 0:64], pst[:, 0:64])
                        nc.vector.tensor_copy(va[:, j, 65:129], pst[:, 64:128])
                    nc.vector.memset(va[:, :, 64:65], 1.0)
                    nc.vector.memset(va[:, :, 129:130], 1.0)
                    vaugs.append(va)

                # ---- attention: both heads interleaved ----
                ctxT = ctxp.tile([128, S], f16, tag="ctxT")
                for qh in range(S // QH):
                    q0 = qh * QH
                    ctx_ps0 = psC.tile([65, QH], f32, tag="ctx0")
                    ctx_ps1 = psC.tile([65, QH], f32, tag="ctx1")
                    ctx_ps = [ctx_ps0, ctx_ps1]

                    def ctx_step(kp, ets):
                        for h in range(2):
                            if FP8_CTX:
                                nc.tensor.matmul(
                                    ctx_ps[h][:],
                                    vaugs[kp][:, :, h * 65:h * 65 + 65],
                                    ets[h][:],
                                    start=(kp == 0), stop=(kp == NKP - 1),
                                    perf_mode=MPM.DoubleRow)
                            else:
                                for j in range(2):
                                    ki = 2 * kp + j
                                    nc.tensor.matmul(
                                        ctx_ps[h][:],
                                        vaugs[kp][:, j, h * 65:h * 65 + 65],
                                        ets[h][:, j, :],
                                        start=(ki == 0), stop=(ki == NKT - 1))

                    # software pipeline: score pair [kp] runs back-to-back,
                    # ctx pair [kp-1] fills the exp latency.
                    prev = None
                    for kp in range(NKP):
                        scs, ets = [], []
                        for h in range(2):
                            sc = psS.tile([128, 2 * QH], f32, tag=f"sc{h}")
                            for j in range(2):
                                ki = 2 * kp + j
                                nc.tensor.matmul(
                                    sc[:, j * QH:(j + 1) * QH],
                                    kt[:, ki * 128:(ki + 1) * 128],
                                    qth[h][:, q0:q0 + QH])
                            scs.append(sc)
                        for h in range(2):
                            et = etp.tile([128, 2, QH], et_dt, tag=f"et{h}")
                            nc.scalar.activation(
                                et[:].rearrange("p a b -> p (a b)"),
                                scs[h][:], AF.Exp)
                            ets.append(et)
                        if prev is not None:
                            ctx_step(prev[0], prev[1])
                        prev = (kp, ets)
                    ctx_step(prev[0], prev[1])
                    # normalize: stage psum (data + sums row 64) to SBUF in
                    # one copy so the ctx bank frees immediately, then
                    # normalize entirely from SBUF off the critical path.
                    for h in range(2):
                        hp = h * 64
                        stg = normp.tile([128, QH], f32, tag=f"stg{h}")
                        nc.vector.tensor_copy(stg[0:65, :], ctx_ps[h][0:65, :])
                        r0 = normp.tile([1, QH], f32, tag="r0")
                        nc.gpsimd.dma_start(r0[:], stg[64:65, :])
                        bcs = normp.tile([64, QH], f32, tag="bcs")
                        nc.gpsimd.partition_broadcast(bcs[:], r0[:])
                        bc = normp.tile([64, QH], f32, tag="bc")
                        scr = normp.tile([64, QH], f32, tag="scr")
                        nc.vector.reciprocal_approx_accurate(
                            bc[:], bcs[:], scratch=scr[:])
                        nc.gpsimd.tensor_mul(
                            out=ctxT[hp:hp + 64, q0:q0 + QH],
                            in0=stg[0:64, :], in1=bc[:])

                # ---- out projection: out[s0+st*128 ...] = ctx @ Wo_slice ----
                for st in range(S // 128):
                    for c in range(D // CH):
                        ps = psP.tile([128, CH], f32, tag="pp")
                        nc.tensor.matmul(ps[:],
                                         ctxT[:, st * 128:(st + 1) * 128],
                                         wo_t[:, c * CH:(c + 1) * CH])
                        ot = ostp.tile([128, CH], f16, tag="ost")
                        nc.vector.tensor_copy(ot[:], ps[:])
                        nc.sync.dma_start(
                            out_d[s0 + st * 128:s0 + (st + 1) * 128,
                                  c * CH:(c + 1) * CH], ot[:])

    nc.compile()
    return nc


def _get_nc():
    if "nc" not in _cache:
        _cache["nc"] = _build()
    return _cache["nc"]


def _in_maps(x, Wq, bq, Wk, bk, Wv, bv, Wo):
    x = np.ascontiguousarray(np.asarray(x, dtype=np.float32))
    xt = np.ascontiguousarray(x.reshape(BS, D).T).astype(np.float16)
    idt = np.eye(128, dtype=np.float16)

    in_maps = []
    for c in range(NCORES):
        sl = slice(c * HSLICE, (c + 1) * HSLICE)
        in_maps.append({
            "xt": xt,
            "wq": (np.ascontiguousarray(np.asarray(Wq, np.float32)[:, sl]) / 8.0).astype(np.float16),
            "wk": np.ascontiguousarray(np.asarray(Wk, np.float32)[:, sl]).astype(np.float16),
            "wv": np.ascontiguousarray(np.asarray(Wv, np.float32)[:, sl]).astype(np.float16),
            "bq": (np.asarray(bq, np.float32)[sl] / 8.0).reshape(HSLICE, 1),
            "bk": np.asarray(bk, np.float32)[sl].reshape(HSLICE, 1),
            "bv": np.asarray(bv, np.float32)[sl].reshape(HSLICE, 1),
            "wo": np.ascontiguousarray(np.asarray(Wo, np.float32)[sl, :]).astype(np.float16),
            "idt": idt,
        })
    return in_maps


def kernel(x, Wq, bq, Wk, bk, Wv, bv, Wo, bo):
    from concourse.bass_utils import run_bass_kernel_spmd

    nc = _get_nc()
    in_maps = _in_maps(x, Wq, bq, Wk, bk, Wv, bv, Wo)

    res = run_bass_kernel_spmd(nc, in_maps, core_ids=list(range(NCORES)),
                               trace=bool(int(os.environ.get("KTRACE", "0"))))
    _cache["last_result"] = res
    acc = res.results[0]["out"].astype(np.float32)
    for c in range(1, NCORES):
        acc += res.results[c]["out"].astype(np.float32)
    acc += np.asarray(bo, np.float32)[None, :]
    return acc.reshape(B, S, D)


# revision 11
# speedup vs baseline: 1.3561x; 1.3561x over previous
"""Multi-head attention forward on 8 TRN2 NeuronCores.

Problem: x[2,2048,1024] @ {Wq,Wk,Wv}[1024,1024] (+bias) -> 16 heads of 64,
softmax(QK^T/8)V per head, concat -> @Wo[1024,1024] + bo.

Sharding: tensor-parallel over d_hid. Core c owns 2 heads (128 dims):
  - computes Q^T,K^T,V^T slices [128, 2048] per batch from full x^T
  - attention for its (2 batches x 2 heads)
  - partial out = ctx_slice @ Wo[slice_rows] -> [4096, 1024] (f16)
Host sums the 8 partials and adds bo (pure reduction, no collectives).

Key layout/perf decisions:
  - x^T [1024, 4096] uploaded pre-transposed, f16, loaded ONCE for both
    batches (8MB SBUF, 1MB strips alternating sync/scalar DMA queues).
  - Weights uploaded pre-tiled [128, 8, 128] (one DMA per matrix).
  - Wq, bq pre-scaled by 1/8 on host (folds the softmax scale).
  - Everything f16 on the PE (no fp32 matmuls): projections, scores,
    V-transposes, ctx, out-projection.
  - Scores computed transposed (S^T[k,q]); softmax normalization comes
    from a ones-column augmented V (row 64 of the ctx psum = row sums).
  - Attention runs a flat software pipeline over (qh, kp): the ctx
    accumulation lags scores/exp by one k-pair, and each qh's
    normalization is split into a DVE phase (stage + reciprocal) and a
    deferred tensor phase (denominator broadcast by a 1-partition f32r
    matmul) so neither in-order queue ever parks.
  - out partials staged on DVE as f16, written with merged 2KB-row DMAs.
  - PSUM banks: scores h0/h1 (2+2) + ctx h0/h1 (1+1) + shared pool for
    proj/transpose/outproj/broadcast (2) = 8.
"""

import os
import numpy as np

B, S, D = 2, 2048, 1024
NCORES = 8
HSLICE = D // NCORES          # 128 = 2 heads x 64
KT_PROJ = D // 128            # 8 contraction tiles for projections
NKT = S // 128                # 16 k-tiles per batch for attention
NKP = NKT // 2                # 8 k-tile pairs
QH = 512                      # q chunk (1 PSUM bank)
CH = 512                      # matmul free-dim chunk
BS = B * S

_cache = {}


def _build():
    import concourse.bacc as bacc
    import concourse.tile as tile
    from concourse import mybir

    f32 = mybir.dt.float32
    f32r = mybir.dt.float32r
    f16 = mybir.dt.float16
    AF = mybir.ActivationFunctionType

    nc = bacc.Bacc("TRN2", target_bir_lowering=False, debug=False,
                   num_devices=NCORES)

    xt_d = nc.dram_tensor("xt", [D, BS], f16, kind="ExternalInput").ap()
    wq_d = nc.dram_tensor("wq", [128, KT_PROJ * HSLICE], f16, kind="ExternalInput").ap()
    wk_d = nc.dram_tensor("wk", [128, KT_PROJ * HSLICE], f16, kind="ExternalInput").ap()
    wv_d = nc.dram_tensor("wv", [128, KT_PROJ * HSLICE], f16, kind="ExternalInput").ap()
    bq_d = nc.dram_tensor("bq", [HSLICE, 1], f32, kind="ExternalInput").ap()
    bk_d = nc.dram_tensor("bk", [HSLICE, 1], f32, kind="ExternalInput").ap()
    bv_d = nc.dram_tensor("bv", [HSLICE, 1], f32, kind="ExternalInput").ap()
    wo_d = nc.dram_tensor("wo", [HSLICE, D], f16, kind="ExternalInput").ap()
    idt_d = nc.dram_tensor("idt", [128, 128], f16, kind="ExternalInput").ap()
    out_d = nc.dram_tensor("out", [BS, D], f16, kind="ExternalOutput").ap()

    with tile.TileContext(nc) as tc:
        with (
            tc.tile_pool(name="wpool", bufs=1) as wpool,
            tc.tile_pool(name="xt", bufs=1) as xtp,
            tc.tile_pool(name="qk", bufs=2) as qkp,
            tc.tile_pool(name="vtmp", bufs=2) as vtp,
            tc.tile_pool(name="vaug", bufs=2) as vap,
            tc.tile_pool(name="et", bufs=2) as etp,
            tc.tile_pool(name="ctx", bufs=2) as ctxp,
            tc.tile_pool(name="norm", bufs=2) as normp,
            tc.tile_pool(name="ost", bufs=3) as ostp,
            tc.tile_pool(name="psS", bufs=1, space="PSUM") as psS,
            tc.tile_pool(name="psC", bufs=1, space="PSUM") as psC,
            tc.tile_pool(name="psP", bufs=2, space="PSUM") as psP,
        ):
            # ---- constants / weights (pre-tiled: one DMA per matrix) ----
            wq_t = wpool.tile([128, KT_PROJ, HSLICE], f16, tag="wq")
            nc.scalar.dma_start(wq_t[:].rearrange("p a b -> p (a b)"), wq_d[:])
            wk_t = wpool.tile([128, KT_PROJ, HSLICE], f16, tag="wk")
            nc.scalar.dma_start(wk_t[:].rearrange("p a b -> p (a b)"), wk_d[:])
            wv_t = wpool.tile([128, KT_PROJ, HSLICE], f16, tag="wv")
            nc.scalar.dma_start(wv_t[:].rearrange("p a b -> p (a b)"), wv_d[:])
            wo_t = wpool.tile([128, D], f16, tag="wo")
            nc.scalar.dma_start(wo_t[:], wo_d[:])
            idt = wpool.tile([128, 128], f16, tag="idt")
            nc.scalar.dma_start(idt[:], idt_d[:])
            bq_t = wpool.tile([128, 1], f32, tag="bq")
            nc.scalar.dma_start(bq_t[:], bq_d[:])
            bk_t = wpool.tile([128, 1], f32, tag="bk")
            nc.scalar.dma_start(bk_t[:], bk_d[:])
            bv_t = wpool.tile([128, 1], f32, tag="bv")
            nc.scalar.dma_start(bv_t[:], bv_d[:])
            ones_t = wpool.tile([128, 64], f16, tag="ones")
            nc.vector.memset(ones_t[:], 1.0)

            # ---- load x^T for BOTH batches, column-strip-major, two queues ----
            xts = []
            for ki in range(KT_PROJ):
                t = xtp.tile([128, BS], f16, tag=f"xt{ki}")
                xts.append(t)
            XC = 1024
            for c in range(BS // XC):
                eng = nc.sync if (c % 2 == 0) else nc.scalar
                for ki in range(KT_PROJ):
                    eng.dma_start(
                        xts[ki][:, c * XC:(c + 1) * XC],
                        xt_d[ki * 128:(ki + 1) * 128, c * XC:(c + 1) * XC])

            for b in range(B):
                s0 = b * S
                # ---- projections ----
                qt0 = qkp.tile([128, S], f16, tag="qt0")
                qt1 = qkp.tile([128, S], f16, tag="qt1")
                qth = [qt0, qt1]
                nc.vector.memset(qt0[64:128, :], 0.0)
                nc.vector.memset(qt1[0:64, :], 0.0)
                kt = qkp.tile([128, S], f16, tag="kt")
                vt = vtp.tile([128, S], f16, tag="vt")
                for di, (dst, w_t, b_t) in enumerate(
                        ((None, wq_t, bq_t), (kt, wk_t, bk_t), (vt, wv_t, bv_t))):
                    for c in range(S // CH):
                        ps = psP.tile([128, CH], f32, tag="pp")
                        for ki in range(KT_PROJ):
                            nc.tensor.matmul(ps[:], w_t[:, ki, :],
                                             xts[ki][:, s0 + c * CH:s0 + (c + 1) * CH],
                                             start=(ki == 0),
                                             stop=(ki == KT_PROJ - 1))
                        if dst is None:
                            nc.vector.tensor_scalar_add(
                                qt0[0:64, c * CH:(c + 1) * CH],
                                ps[0:64, :], b_t[0:64, 0:1])
                            nc.vector.tensor_scalar_add(
                                qt1[64:128, c * CH:(c + 1) * CH],
                                ps[64:128, :], b_t[64:128, 0:1])
                        else:
                            nc.vector.tensor_scalar_add(
                                dst[:, c * CH:(c + 1) * CH], ps[:], b_t[:, 0:1])

                # ---- V^T -> V_aug pair tiles [128, 2, 130] ----
                vaugs = []
                for kp in range(NKP):
                    va = vap.tile([128, 2, 130], f16, tag=f"va{kp}")
                    for j in range(2):
                        ki = 2 * kp + j
                        pst = psP.tile([128, 128], f16, tag="pp")
                        nc.tensor.transpose(pst[:],
                                            vt[:, ki * 128:(ki + 1) * 128],
                                            idt[:])
                        nc.vector.tensor_copy(va[:, j, 0:64], pst[:, 0:64])
                        nc.vector.tensor_copy(va[:, j, 65:129], pst[:, 64:128])
                    nc.vector.memset(va[:, :, 64:65], 1.0)
                    nc.vector.memset(va[:, :, 129:130], 1.0)
                    vaugs.append(va)

                # ---- attention: flat (qh, kp) software pipeline ----
                ctxT = ctxp.tile([128, S], f16, tag="ctxT")

                def ctx_step(ctx_ps, vaugs_, kp, ets):
                    for h in range(2):
                        for j in range(2):
                            ki = 2 * kp + j
                            nc.tensor.matmul(
                                ctx_ps[h][:],
                                vaugs_[kp][:, j, h * 65:h * 65 + 65],
                                ets[h][:, j, :],
                                start=(ki == 0), stop=(ki == NKT - 1))

                def norm_phase1(ctx_ps, q0):
                    # stage psum data rows to SBUF f32 and the sums row to a
                    # partition-0 f16 row tile (DVE only; frees ctx banks).
                    stgs, rs = [], []
                    for h in range(2):
                        stg = normp.tile([128, QH], f32, tag=f"stg{h}")
                        nc.vector.tensor_copy(stg[0:64, :], ctx_ps[h][0:64, :])
                        stgs.append(stg)
                    for h in range(2):
                        r = normp.tile([1, QH], f16, tag=f"r{h}")
                        nc.vector.tensor_copy(r[0:1, :], ctx_ps[h][64:65, :])
                        rs.append(r)
                    return stgs, rs

                def norm_phase2(stgs, rs, q0):
                    # broadcast the sums row across 64 partitions via a
                    # 1-partition f16 matmul, reciprocal on the psum result,
                    # then scale ctx rows (DVE).
                    for h in range(2):
                        bcps = psP.tile([64, QH], f32, tag="pp")
                        nc.tensor.matmul(bcps[:], ones_t[0:1, :],
                                         rs[h][0:1, :])
                        bc = normp.tile([64, QH], f32, tag=f"bc{h}")
                        nc.vector.reciprocal_approx_fast(bc[:], bcps[:])
                        nc.vector.tensor_mul(
                            out=ctxT[h * 64:h * 64 + 64, q0:q0 + QH],
                            in0=stgs[h][0:64, :], in1=bc[:])

                prev = None        # (ctx_ps, vaugs, kp, ets, q0)
                pend2 = None       # deferred norm_phase2 closure
                for qh in range(S // QH):
                    q0 = qh * QH
                    ctx_ps0 = psC.tile([65, QH], f32, tag="ctx0")
                    ctx_ps1 = psC.tile([65, QH], f32, tag="ctx1")
                    ctx_ps = [ctx_ps0, ctx_ps1]
                    for kp in range(NKP):
                        scs, ets = [], []
                        for h in range(2):
                            sc = psS.tile([128, 2 * QH], f32, tag=f"sc{h}")
                            for j in range(2):
                                ki = 2 * kp + j
                                nc.tensor.matmul(
                                    sc[:, j * QH:(j + 1) * QH],
                                    kt[:, ki * 128:(ki + 1) * 128],
                                    qth[h][:, q0:q0 + QH])
                            scs.append(sc)
                        for h in range(2):
                            et = etp.tile([128, 2, QH], f16, tag=f"et{h}")
                            nc.scalar.activation(
                                et[:].rearrange("p a b -> p (a b)"),
                                scs[h][:], AF.Exp)
                            ets.append(et)
                        if pend2 is not None:
                            pend2()
                            pend2 = None
                        if prev is not None:
                            ctx_step(*prev[:4])
                            if prev[2] == NKP - 1:
                                stgs, rs = norm_phase1(prev[0], prev[4])
                                pend2 = (lambda st=stgs, rr=rs, qq=prev[4]:
                                         norm_phase2(st, rr, qq))
                        prev = (ctx_ps, vaugs, kp, ets, q0)
                # drain the pipeline for this batch
                if pend2 is not None:
                    pend2()
                    pend2 = None
                ctx_step(*prev[:4])
                stgs, rs = norm_phase1(prev[0], prev[4])
                norm_phase2(stgs, rs, prev[4])
                prev = None

                # ---- out projection ----
                for st in range(S // 128):
                    ot = ostp.tile([128, D], f16, tag="ost")
                    for c in range(D // CH):
                        ps = psP.tile([128, CH], f32, tag="pp")
                        nc.tensor.matmul(ps[:],
                                         ctxT[:, st * 128:(st + 1) * 128],
                                         wo_t[:, c * CH:(c + 1) * CH])
                        nc.vector.tensor_copy(ot[:, c * CH:(c + 1) * CH], ps[:])
                    nc.sync.dma_start(
                        out_d[s0 + st * 128:s0 + (st + 1) * 128, :], ot[:])

    nc.compile()
    return nc


def _get_nc():
    if "nc" not in _cache:
        _cache["nc"] = _build()
    return _cache["nc"]


def _tile_w(w):
    # [1024, 128] -> [128, 8, 128] -> flat [128, 1024] (partition-major tiles)
    return np.ascontiguousarray(
        w.reshape(KT_PROJ, 128, HSLICE).transpose(1, 0, 2).reshape(128, -1))


def _in_maps(x, Wq, bq, Wk, bk, Wv, bv, Wo):
    x = np.ascontiguousarray(np.asarray(x, dtype=np.float32))
    xt = np.ascontiguousarray(x.reshape(BS, D).T).astype(np.float16)
    idt = np.eye(128, dtype=np.float16)

    in_maps = []
    for c in range(NCORES):
        sl = slice(c * HSLICE, (c + 1) * HSLICE)
        wq = (np.asarray(Wq, np.float32)[:, sl] / 8.0).astype(np.float16)
        wk = np.asarray(Wk, np.float32)[:, sl].astype(np.float16)
        wv = np.asarray(Wv, np.float32)[:, sl].astype(np.float16)
        in_maps.append({
            "xt": xt,
            "wq": _tile_w(wq),
            "wk": _tile_w(wk),
            "wv": _tile_w(wv),
            "bq": (np.asarray(bq, np.float32)[sl] / 8.0).reshape(HSLICE, 1),
            "bk": np.asarray(bk, np.float32)[sl].reshape(HSLICE, 1),
            "bv": np.asarray(bv, np.float32)[sl].reshape(HSLICE, 1),
            "wo": np.ascontiguousarray(np.asarray(Wo, np.float32)[sl, :]).astype(np.float16),
            "idt": idt,
        })
    return in_maps


def kernel(x, Wq, bq, Wk, bk, Wv, bv, Wo, bo):
    from concourse.bass_utils import run_bass_kernel_spmd

    nc = _get_nc()
    in_maps = _in_maps(x, Wq, bq, Wk, bk, Wv, bv, Wo)

    res = run_bass_kernel_spmd(nc, in_maps, core_ids=list(range(NCORES)),
                               trace=bool(int(os.environ.get("KTRACE", "0"))))
    _cache["last_result"] = res
    acc = res.results[0]["out"].astype(np.float32)
    for c in range(1, NCORES):
        acc += res.results[c]["out"].astype(np.float32)
    acc += np.asarray(bo, np.float32)[None, :]
    return acc.reshape(B, S, D)


# revision 16
# speedup vs baseline: 1.5563x; 1.1476x over previous
"""Multi-head attention forward on 8 TRN2 NeuronCores.

Problem: x[2,2048,1024] @ {Wq,Wk,Wv}[1024,1024] (+bias) -> 16 heads of 64,
softmax(QK^T/8)V per head, concat -> @Wo[1024,1024] + bo.

Sharding: tensor-parallel over d_hid. Core c owns 2 heads (128 dims):
  - computes Q^T,K^T,V^T slices [128, 2048] per batch from full x^T
  - attention for its (2 batches x 2 heads)
  - partial out = ctx_slice @ Wo[slice_rows] -> [4096, 1024] (f16)
Host sums the 8 partials and adds bo (pure reduction, no collectives).

Key layout/perf decisions:
  - x^T [1024, 4096] uploaded pre-transposed, f16, loaded ONCE for both
    batches (8MB SBUF, 1MB strips alternating sync/scalar DMA queues).
  - Weights uploaded pre-tiled [128, 8, 128] (one DMA per matrix).
  - Wq, bq pre-scaled by 1/8 on host (folds the softmax scale).
  - Everything f16 on the PE (no fp32 matmuls): projections, scores,
    V-transposes, ctx, out-projection.
  - Scores computed transposed (S^T[k,q]); softmax normalization comes
    from a ones-column augmented V (row 64 of the ctx psum = row sums).
  - Attention runs a flat software pipeline over (qh, kp): the ctx
    accumulation lags scores/exp by one k-pair, and each qh's
    normalization is split into a DVE phase (stage + reciprocal) and a
    deferred tensor phase (denominator broadcast by a 1-partition f32r
    matmul) so neither in-order queue ever parks.
  - out partials staged on DVE as f16, written with merged 2KB-row DMAs.
  - PSUM banks: scores h0/h1 (2+2) + ctx h0/h1 (1+1) + shared pool for
    proj/transpose/outproj/broadcast (2) = 8.
"""

import os
import numpy as np

B, S, D = 2, 2048, 1024
NCORES = 8
HSLICE = D // NCORES          # 128 = 2 heads x 64
KT_PROJ = D // 128            # 8 contraction tiles for projections
NKT = S // 128                # 16 k-tiles per batch for attention
NKP = NKT // 2                # 8 k-tile pairs
QH = 512                      # q chunk (1 PSUM bank)
CH = 512                      # matmul free-dim chunk
BS = B * S

_cache = {}


def _build():
    import concourse.bacc as bacc
    import concourse.tile as tile
    from concourse import mybir

    f32 = mybir.dt.float32
    f32r = mybir.dt.float32r
    f16 = mybir.dt.float16
    AF = mybir.ActivationFunctionType

    nc = bacc.Bacc("TRN2", target_bir_lowering=False, debug=False,
                   num_devices=NCORES)

    xt_d = nc.dram_tensor("xt", [D, BS], f16, kind="ExternalInput").ap()
    wq_d = nc.dram_tensor("wq", [128, KT_PROJ * HSLICE], f16, kind="ExternalInput").ap()
    wk_d = nc.dram_tensor("wk", [128, KT_PROJ * HSLICE], f16, kind="ExternalInput").ap()
    wv_d = nc.dram_tensor("wv", [128, KT_PROJ * HSLICE], f16, kind="ExternalInput").ap()
    bq_d = nc.dram_tensor("bq", [HSLICE, 1], f32, kind="ExternalInput").ap()
    bk_d = nc.dram_tensor("bk", [HSLICE, 1], f32, kind="ExternalInput").ap()
    bv_d = nc.dram_tensor("bv", [HSLICE, 1], f32, kind="ExternalInput").ap()
    wo_d = nc.dram_tensor("wo", [HSLICE, D], f16, kind="ExternalInput").ap()
    idt_d = nc.dram_tensor("idt", [128, 128], f16, kind="ExternalInput").ap()
    out_d = nc.dram_tensor("out", [BS, D], f16, kind="ExternalOutput").ap()

    with tile.TileContext(nc) as tc:
        with (
            tc.tile_pool(name="wpool", bufs=1) as wpool,
            tc.tile_pool(name="xt", bufs=1) as xtp,
            tc.tile_pool(name="qk", bufs=2) as qkp,
            tc.tile_pool(name="vtmp", bufs=2) as vtp,
            tc.tile_pool(name="vaug", bufs=2) as vap,
            tc.tile_pool(name="et", bufs=2) as etp,
            tc.tile_pool(name="ctx", bufs=2) as ctxp,
            tc.tile_pool(name="norm", bufs=2) as normp,
            tc.tile_pool(name="ost", bufs=3) as ostp,
            tc.tile_pool(name="psS", bufs=1, space="PSUM") as psS,
            tc.tile_pool(name="psC", bufs=1, space="PSUM") as psC,
            tc.tile_pool(name="psP", bufs=2, space="PSUM") as psP,
        ):
            # ---- constants / weights (pre-tiled, gpsimd queue, by first use) ----
            wq_t = wpool.tile([128, KT_PROJ, HSLICE], f16, tag="wq")
            nc.gpsimd.dma_start(wq_t[:].rearrange("p a b -> p (a b)"), wq_d[:])
            bq_t = wpool.tile([128, 1], f32, tag="bq")
            nc.gpsimd.dma_start(bq_t[:], bq_d[:])
            bk_t = wpool.tile([128, 1], f32, tag="bk")
            nc.gpsimd.dma_start(bk_t[:], bk_d[:])
            bv_t = wpool.tile([128, 1], f32, tag="bv")
            nc.gpsimd.dma_start(bv_t[:], bv_d[:])
            wk_t = wpool.tile([128, KT_PROJ, HSLICE], f16, tag="wk")
            nc.gpsimd.dma_start(wk_t[:].rearrange("p a b -> p (a b)"), wk_d[:])
            wv_t = wpool.tile([128, KT_PROJ, HSLICE], f16, tag="wv")
            nc.gpsimd.dma_start(wv_t[:].rearrange("p a b -> p (a b)"), wv_d[:])
            idt = wpool.tile([128, 128], f16, tag="idt")
            nc.gpsimd.dma_start(idt[:], idt_d[:])
            wo_t = wpool.tile([128, D], f16, tag="wo")
            nc.gpsimd.dma_start(wo_t[:], wo_d[:])
            ones_t = wpool.tile([128, 64], f16, tag="ones")
            nc.vector.memset(ones_t[:], 1.0)

            # ---- load x^T for BOTH batches, column-strip-major, three queues ----
            xts = []
            for ki in range(KT_PROJ):
                t = xtp.tile([128, BS], f16, tag=f"xt{ki}")
                xts.append(t)
            XC = 512
            qs = [nc.sync, nc.scalar, nc.gpsimd]
            qi = 0
            for c in range(BS // XC):
                for ki in range(KT_PROJ):
                    qs[qi % 3].dma_start(
                        xts[ki][:, c * XC:(c + 1) * XC],
                        xt_d[ki * 128:(ki + 1) * 128, c * XC:(c + 1) * XC])
                    qi += 1

            for b in range(B):
                s0 = b * S
                # ---- projections ----
                qt0 = qkp.tile([128, S], f16, tag="qt0")
                qt1 = qkp.tile([128, S], f16, tag="qt1")
                qth = [qt0, qt1]
                nc.vector.memset(qt0[64:128, :], 0.0)
                nc.vector.memset(qt1[0:64, :], 0.0)
                kt = qkp.tile([128, S], f16, tag="kt")
                vt = vtp.tile([128, S], f16, tag="vt")
                for di, (dst, w_t, b_t) in enumerate(
                        ((None, wq_t, bq_t), (kt, wk_t, bk_t), (vt, wv_t, bv_t))):
                    for c in range(S // CH):
                        ps = psP.tile([128, CH], f32, tag="pp")
                        for ki in range(KT_PROJ):
                            nc.tensor.matmul(ps[:], w_t[:, ki, :],
                                             xts[ki][:, s0 + c * CH:s0 + (c + 1) * CH],
                                             start=(ki == 0),
                                             stop=(ki == KT_PROJ - 1))
                        if dst is None:
                            nc.vector.tensor_scalar_add(
                                qt0[0:64, c * CH:(c + 1) * CH],
                                ps[0:64, :], b_t[0:64, 0:1])
                            nc.vector.tensor_scalar_add(
                                qt1[64:128, c * CH:(c + 1) * CH],
                                ps[64:128, :], b_t[64:128, 0:1])
                        else:
                            nc.vector.tensor_scalar_add(
                                dst[:, c * CH:(c + 1) * CH], ps[:], b_t[:, 0:1])

                # ---- V^T -> V_aug pair tiles [128, 2, 130] ----
                vaugs = []
                for kp in range(NKP):
                    va = vap.tile([128, 2, 130], f16, tag=f"va{kp}")
                    for j in range(2):
                        ki = 2 * kp + j
                        pst = psP.tile([128, 128], f16, tag="pp")
                        nc.tensor.transpose(pst[:],
                                            vt[:, ki * 128:(ki + 1) * 128],
                                            idt[:])
                        nc.vector.tensor_copy(va[:, j, 0:64], pst[:, 0:64])
                        nc.vector.tensor_copy(va[:, j, 65:129], pst[:, 64:128])
                    nc.vector.memset(va[:, :, 64:65], 1.0)
                    nc.vector.memset(va[:, :, 129:130], 1.0)
                    vaugs.append(va)

                # ---- attention: flat (qh, kp) software pipeline with
                # deferred norm-phase2 / out-projection work drained one
                # unit per kp step (keeps every queue busy, no tail) ----
                ctxT = ctxp.tile([128, S], f16, tag="ctxT")
                pending = []

                def outproj_st(st):
                    ot = ostp.tile([128, D], f16, tag="ost")
                    for c2 in range(D // CH):
                        ps = psP.tile([128, CH], f32, tag="pp")
                        nc.tensor.matmul(ps[:],
                                         ctxT[:, st * 128:(st + 1) * 128],
                                         wo_t[:, c2 * CH:(c2 + 1) * CH])
                        nc.vector.tensor_copy(ot[:, c2 * CH:(c2 + 1) * CH],
                                              ps[:])
                    nc.sync.dma_start(
                        out_d[s0 + st * 128:s0 + (st + 1) * 128, :], ot[:])

                def ctx_step(ctx_ps, vaugs_, kp, ets):
                    for h in range(2):
                        for j in range(2):
                            ki = 2 * kp + j
                            nc.tensor.matmul(
                                ctx_ps[h][:],
                                vaugs_[kp][:, j, h * 65:h * 65 + 65],
                                ets[h][:, j, :],
                                start=(ki == 0), stop=(ki == NKT - 1))

                def norm_phase1(ctx_ps, q0):
                    # stage psum data rows to SBUF f32 and the sums row to a
                    # partition-0 f16 row tile (DVE only; frees ctx banks).
                    stgs, rs = [], []
                    for h in range(2):
                        stg = normp.tile([128, QH], f32, tag=f"stg{h}")
                        nc.vector.tensor_copy(stg[0:64, :], ctx_ps[h][0:64, :])
                        stgs.append(stg)
                    for h in range(2):
                        r = normp.tile([1, QH], f16, tag=f"r{h}")
                        nc.vector.tensor_copy(r[0:1, :], ctx_ps[h][64:65, :])
                        rs.append(r)
                    return stgs, rs

                def norm_phase2(stgs, rs, q0):
                    # broadcast the sums row across 64 partitions via a
                    # 1-partition f16 matmul, reciprocal on the psum result,
                    # then scale ctx rows (DVE). Enqueues this q-chunk's
                    # out-projection tiles as deferred work.
                    for h in range(2):
                        bcps = psP.tile([64, QH], f32, tag="pp")
                        nc.tensor.matmul(bcps[:], ones_t[0:1, :],
                                         rs[h][0:1, :])
                        bc = normp.tile([64, QH], f32, tag=f"bc{h}")
                        nc.vector.reciprocal_approx_fast(bc[:], bcps[:])
                        nc.vector.tensor_mul(
                            out=ctxT[h * 64:h * 64 + 64, q0:q0 + QH],
                            in0=stgs[h][0:64, :], in1=bc[:])
                    for stq in range(QH // 128):
                        pending.append(
                            lambda s_=q0 // 128 + stq: outproj_st(s_))

                prev = None        # (ctx_ps, vaugs, kp, ets, q0)
                for qh in range(S // QH):
                    q0 = qh * QH
                    ctx_ps0 = psC.tile([65, QH], f32, tag="ctx0")
                    ctx_ps1 = psC.tile([65, QH], f32, tag="ctx1")
                    ctx_ps = [ctx_ps0, ctx_ps1]
                    for kp in range(NKP):
                        scs, ets = [], []
                        for h in range(2):
                            sc = psS.tile([128, 2 * QH], f32, tag=f"sc{h}")
                            for j in range(2):
                                ki = 2 * kp + j
                                nc.tensor.matmul(
                                    sc[:, j * QH:(j + 1) * QH],
                                    kt[:, ki * 128:(ki + 1) * 128],
                                    qth[h][:, q0:q0 + QH])
                            scs.append(sc)
                        for h in range(2):
                            et = etp.tile([128, 2, QH], f16, tag=f"et{h}")
                            nc.scalar.activation(
                                et[:].rearrange("p a b -> p (a b)"),
                                scs[h][:], AF.Exp)
                            ets.append(et)
                        if pending:
                            pending.pop(0)()
                        if prev is not None:
                            ctx_step(*prev[:4])
                            if prev[2] == NKP - 1:
                                stgs, rs = norm_phase1(prev[0], prev[4])
                                pending.append(
                                    lambda st=stgs, rr=rs, qq=prev[4]:
                                    norm_phase2(st, rr, qq))
                        prev = (ctx_ps, vaugs, kp, ets, q0)
                # drain the pipeline for this batch
                ctx_step(*prev[:4])
                stgs, rs = norm_phase1(prev[0], prev[4])
                while pending:
                    pending.pop(0)()
                norm_phase2(stgs, rs, prev[4])
                while pending:
                    pending.pop(0)()
                prev = None

    nc.compile()
    return nc


def _get_nc():
    if "nc" not in _cache:
        _cache["nc"] = _build()
    return _cache["nc"]


def _tile_w(w):
    # [1024, 128] -> [128, 8, 128] -> flat [128, 1024] (partition-major tiles)
    return np.ascontiguousarray(
        w.reshape(KT_PROJ, 128, HSLICE).transpose(1, 0, 2).reshape(128, -1))


def _in_maps(x, Wq, bq, Wk, bk, Wv, bv, Wo):
    x = np.ascontiguousarray(np.asarray(x, dtype=np.float32))
    xt = np.ascontiguousarray(x.reshape(BS, D).T).astype(np.float16)
    idt = np.eye(128, dtype=np.float16)

    in_maps = []
    for c in range(NCORES):
        sl = slice(c * HSLICE, (c + 1) * HSLICE)
        wq = (np.asarray(Wq, np.float32)[:, sl] / 8.0).astype(np.float16)
        wk = np.asarray(Wk, np.float32)[:, sl].astype(np.float16)
        wv = np.asarray(Wv, np.float32)[:, sl].astype(np.float16)
        in_maps.append({
            "xt": xt,
            "wq": _tile_w(wq),
            "wk": _tile_w(wk),
            "wv": _tile_w(wv),
            "bq": (np.asarray(bq, np.float32)[sl] / 8.0).reshape(HSLICE, 1),
            "bk": np.asarray(bk, np.float32)[sl].reshape(HSLICE, 1),
            "bv": np.asarray(bv, np.float32)[sl].reshape(HSLICE, 1),
            "wo": np.ascontiguousarray(np.asarray(Wo, np.float32)[sl, :]).astype(np.float16),
            "idt": idt,
        })
    return in_maps


def kernel(x, Wq, bq, Wk, bk, Wv, bv, Wo, bo):
    from concourse.bass_utils import run_bass_kernel_spmd

    nc = _get_nc()
    in_maps = _in_maps(x, Wq, bq, Wk, bk, Wv, bv, Wo)

    res = run_bass_kernel_spmd(nc, in_maps, core_ids=list(range(NCORES)),
                               trace=bool(int(os.environ.get("KTRACE", "0"))))
    _cache["last_result"] = res
    acc = res.results[0]["out"].astype(np.float32)
    for c in range(1, NCORES):
        acc += res.results[c]["out"].astype(np.float32)
    acc += np.asarray(bo, np.float32)[None, :]
    return acc.reshape(B, S, D)


# revision 20
# speedup vs baseline: 1.6317x; 1.0485x over previous
"""Multi-head attention forward on 8 TRN2 NeuronCores.

Problem: x[2,2048,1024] @ {Wq,Wk,Wv}[1024,1024] (+bias) -> 16 heads of 64,
softmax(QK^T/8)V per head, concat -> @Wo[1024,1024] + bo.

Sharding: tensor-parallel over d_hid. Core c owns 2 heads (128 dims):
  - computes Q^T,K^T,V^T slices [128, 2048] per batch from full x^T
  - attention for its (2 batches x 2 heads)
  - partial out = ctx_slice @ Wo[slice_rows] -> [4096, 1024] (f16)
Host sums the 8 partials and adds bo (pure reduction, no collectives).

Key layout/perf decisions:
  - x^T [1024, 4096] uploaded pre-transposed, f16, loaded ONCE for both
    batches (8MB SBUF, 1MB strips alternating sync/scalar DMA queues).
  - Weights uploaded pre-tiled [128, 8, 128] (one DMA per matrix).
  - Wq, bq pre-scaled by 1/8 on host (folds the softmax scale).
  - Everything f16 on the PE (no fp32 matmuls): projections, scores,
    V-transposes, ctx, out-projection.
  - Scores computed transposed (S^T[k,q]); softmax normalization comes
    from a ones-column augmented V (row 64 of the ctx psum = row sums).
  - Attention runs a flat software pipeline over (qh, kp): the ctx
    accumulation lags scores/exp by one k-pair, and each qh's
    normalization is split into a DVE phase (stage + reciprocal) and a
    deferred tensor phase (denominator broadcast by a 1-partition f32r
    matmul) so neither in-order queue ever parks.
  - out partials staged on DVE as f16, written with merged 2KB-row DMAs.
  - PSUM banks: scores h0/h1 (2+2) + ctx h0/h1 (1+1) + shared pool for
    proj/transpose/outproj/broadcast (2) = 8.
"""

import os
import numpy as np

B, S, D = 2, 2048, 1024
NCORES = 8
HSLICE = D // NCORES          # 128 = 2 heads x 64
KT_PROJ = D // 128            # 8 contraction tiles for projections
NKT = S // 128                # 16 k-tiles per batch for attention
NKP = NKT // 2                # 8 k-tile pairs
QH = 512                      # q chunk (1 PSUM bank)
CH = 512                      # matmul free-dim chunk
BS = B * S

_cache = {}


def _build():
    import concourse.bacc as bacc
    import concourse.tile as tile
    from concourse import mybir

    f32 = mybir.dt.float32
    f32r = mybir.dt.float32r
    f16 = mybir.dt.float16
    AF = mybir.ActivationFunctionType

    nc = bacc.Bacc("TRN2", target_bir_lowering=False, debug=False,
                   num_devices=NCORES)

    xt_d = nc.dram_tensor("xt", [D, BS], f16, kind="ExternalInput").ap()
    w3_d = nc.dram_tensor("w3", [128, 3 * KT_PROJ * HSLICE], f16, kind="ExternalInput").ap()
    b3_d = nc.dram_tensor("b3", [HSLICE, 3], f32, kind="ExternalInput").ap()
    wo_d = nc.dram_tensor("wo", [HSLICE, D], f16, kind="ExternalInput").ap()
    idt_d = nc.dram_tensor("idt", [128, 128], f16, kind="ExternalInput").ap()
    out_d = nc.dram_tensor("out", [BS, D], f16, kind="ExternalOutput").ap()

    with tile.TileContext(nc) as tc:
        with (
            tc.tile_pool(name="wpool", bufs=1) as wpool,
            tc.tile_pool(name="xt", bufs=1) as xtp,
            tc.tile_pool(name="qk", bufs=2) as qkp,
            tc.tile_pool(name="vtmp", bufs=2) as vtp,
            tc.tile_pool(name="vaug", bufs=2) as vap,
            tc.tile_pool(name="et", bufs=2) as etp,
            tc.tile_pool(name="ctx", bufs=2) as ctxp,
            tc.tile_pool(name="norm", bufs=2) as normp,
            tc.tile_pool(name="ost", bufs=3) as ostp,
            tc.tile_pool(name="psS", bufs=1, space="PSUM") as psS,
            tc.tile_pool(name="psC", bufs=1, space="PSUM") as psC,
            tc.tile_pool(name="psP", bufs=2, space="PSUM") as psP,
        ):
            # ---- constants / weights (merged DMAs on the gpsimd queue) ----
            w3_t = wpool.tile([128, 3, KT_PROJ, HSLICE], f16, tag="w3")
            nc.gpsimd.dma_start(w3_t[:].rearrange("p a b c -> p (a b c)"), w3_d[:])
            wq_t, wk_t, wv_t = w3_t[:, 0], w3_t[:, 1], w3_t[:, 2]
            b3_t = wpool.tile([128, 3], f32, tag="b3")
            nc.gpsimd.dma_start(b3_t[:], b3_d[:])
            bq_t, bk_t, bv_t = b3_t[:, 0:1], b3_t[:, 1:2], b3_t[:, 2:3]
            wo_t = wpool.tile([128, D], f16, tag="wo")
            nc.gpsimd.dma_start(wo_t[:], wo_d[:])
            idt = wpool.tile([128, 128], f16, tag="idt")
            nc.gpsimd.dma_start(idt[:], idt_d[:])
            ones_t = wpool.tile([128, 64], f16, tag="ones")
            nc.vector.memset(ones_t[:], 1.0)

            # ---- load x^T for BOTH batches, column-strip-major, three queues ----
            xts = []
            for ki in range(KT_PROJ):
                t = xtp.tile([128, BS], f16, tag=f"xt{ki}")
                xts.append(t)
            XC = 1024
            qs = [nc.sync, nc.scalar, nc.gpsimd]
            qi = 0
            for c in range(BS // XC):
                for ki in range(KT_PROJ):
                    qs[qi % 3].dma_start(
                        xts[ki][:, c * XC:(c + 1) * XC],
                        xt_d[ki * 128:(ki + 1) * 128, c * XC:(c + 1) * XC])
                    qi += 1

            pending = []
            for b in range(B):
                s0 = b * S
                # ---- projections ----
                qt0 = qkp.tile([128, S], f16, tag="qt0")
                qt1 = qkp.tile([128, S], f16, tag="qt1")
                qth = [qt0, qt1]
                nc.vector.memset(qt0[64:128, :], 0.0)
                nc.vector.memset(qt1[0:64, :], 0.0)
                kt = qkp.tile([128, S], f16, tag="kt")
                vt = vtp.tile([128, S], f16, tag="vt")
                for di, (dst, w_t, b_t) in enumerate(
                        ((None, wq_t, bq_t), (kt, wk_t, bk_t), (vt, wv_t, bv_t))):
                    for c in range(S // CH):
                        ps = psP.tile([128, CH], f32, tag="pp")
                        for ki in range(KT_PROJ):
                            nc.tensor.matmul(ps[:], w_t[:, ki, :],
                                             xts[ki][:, s0 + c * CH:s0 + (c + 1) * CH],
                                             start=(ki == 0),
                                             stop=(ki == KT_PROJ - 1))
                        if dst is None:
                            nc.vector.tensor_scalar_add(
                                qt0[0:64, c * CH:(c + 1) * CH],
                                ps[0:64, :], b_t[0:64, :])
                            nc.vector.tensor_scalar_add(
                                qt1[64:128, c * CH:(c + 1) * CH],
                                ps[64:128, :], b_t[64:128, :])
                        else:
                            nc.vector.tensor_scalar_add(
                                dst[:, c * CH:(c + 1) * CH], ps[:], b_t[:, :])

                # ---- V^T -> V_aug pair tiles [128, 2, 130] ----
                vaugs = []
                for kp in range(NKP):
                    va = vap.tile([128, 2, 130], f16, tag=f"va{kp}")
                    for j in range(2):
                        ki = 2 * kp + j
                        pst = psP.tile([128, 128], f16, tag="pp")
                        nc.tensor.transpose(pst[:],
                                            vt[:, ki * 128:(ki + 1) * 128],
                                            idt[:])
                        nc.vector.tensor_copy(va[:, j, 0:64], pst[:, 0:64])
                        nc.vector.tensor_copy(va[:, j, 65:129], pst[:, 64:128])
                    nc.vector.memset(va[:, :, 64:65], 1.0)
                    nc.vector.memset(va[:, :, 129:130], 1.0)
                    vaugs.append(va)

                # ---- attention: flat (qh, kp) software pipeline with
                # deferred norm-phase2 / out-projection work drained one
                # unit per kp step (keeps every queue busy, no tail) ----
                ctxT = ctxp.tile([128, S], f16, tag="ctxT")

                def outproj_st(st, s0=s0, ctxT=ctxT):
                    ot = ostp.tile([128, D], f16, tag="ost")
                    for c2 in range(D // CH):
                        ps = psP.tile([128, CH], f32, tag="pp")
                        nc.tensor.matmul(ps[:],
                                         ctxT[:, st * 128:(st + 1) * 128],
                                         wo_t[:, c2 * CH:(c2 + 1) * CH])
                        nc.vector.tensor_copy(ot[:, c2 * CH:(c2 + 1) * CH],
                                              ps[:])
                    nc.sync.dma_start(
                        out_d[s0 + st * 128:s0 + (st + 1) * 128, :], ot[:])

                def ctx_step(ctx_ps, vaugs_, kp, ets):
                    for h in range(2):
                        for j in range(2):
                            ki = 2 * kp + j
                            nc.tensor.matmul(
                                ctx_ps[h][:],
                                vaugs_[kp][:, j, h * 65:h * 65 + 65],
                                ets[h][:, j, :],
                                start=(ki == 0), stop=(ki == NKT - 1))

                def norm_phase1(ctx_ps, q0):
                    # stage psum data rows to SBUF f32 and the sums row to a
                    # partition-0 f16 row tile (DVE only; frees ctx banks).
                    stgs, rs = [], []
                    for h in range(2):
                        stg = normp.tile([128, QH], f32, tag=f"stg{h}")
                        nc.vector.tensor_copy(stg[0:64, :], ctx_ps[h][0:64, :])
                        stgs.append(stg)
                    for h in range(2):
                        r = normp.tile([1, QH], f16, tag=f"r{h}")
                        nc.vector.tensor_copy(r[0:1, :], ctx_ps[h][64:65, :])
                        rs.append(r)
                    return stgs, rs

                def norm_phase2(stgs, rs, q0, ctxT=ctxT, outproj_st=outproj_st):
                    # broadcast the sums row across 64 partitions via a
                    # 1-partition f16 matmul, reciprocal on the psum result,
                    # then scale ctx rows (DVE). Enqueues this q-chunk's
                    # out-projection tiles as deferred work.
                    for h in range(2):
                        bcps = psP.tile([64, QH], f32, tag="pp")
                        nc.tensor.matmul(bcps[:], ones_t[0:1, :],
                                         rs[h][0:1, :])
                        bc = normp.tile([64, QH], f32, tag=f"bc{h}")
                        nc.vector.reciprocal_approx_fast(bc[:], bcps[:])
                        nc.vector.tensor_mul(
                            out=ctxT[h * 64:h * 64 + 64, q0:q0 + QH],
                            in0=stgs[h][0:64, :], in1=bc[:])
                    for stq in range(QH // 128):
                        pending.append(
                            lambda s_=q0 // 128 + stq, f=outproj_st: f(s_))

                prev = None        # (ctx_ps, vaugs, kp, ets, q0)
                for qh in range(S // QH):
                    q0 = qh * QH
                    ctx_ps0 = psC.tile([65, QH], f32, tag="ctx0")
                    ctx_ps1 = psC.tile([65, QH], f32, tag="ctx1")
                    ctx_ps = [ctx_ps0, ctx_ps1]
                    for kp in range(NKP):
                        scs, ets = [], []
                        for h in range(2):
                            sc = psS.tile([128, 2 * QH], f32, tag=f"sc{h}")
                            for j in range(2):
                                ki = 2 * kp + j
                                nc.tensor.matmul(
                                    sc[:, j * QH:(j + 1) * QH],
                                    kt[:, ki * 128:(ki + 1) * 128],
                                    qth[h][:, q0:q0 + QH])
                            scs.append(sc)
                        for h in range(2):
                            et = etp.tile([128, 2, QH], f16, tag=f"et{h}")
                            nc.scalar.activation(
                                et[:].rearrange("p a b -> p (a b)"),
                                scs[h][:], AF.Exp)
                            ets.append(et)
                        if pending:
                            pending.pop(0)()
                        if prev is not None:
                            ctx_step(*prev[:4])
                            if prev[2] == NKP - 1:
                                stgs, rs = norm_phase1(prev[0], prev[4])
                                pending.append(
                                    lambda st=stgs, rr=rs, qq=prev[4],
                                    f=norm_phase2: f(st, rr, qq))
                        prev = (ctx_ps, vaugs, kp, ets, q0)
                # flush this batch's last ctx/norm; leftover deferred units
                # drain inside the next batch's attention (or below if last)
                ctx_step(*prev[:4])
                stgs, rs = norm_phase1(prev[0], prev[4])
                pending.append(lambda st=stgs, rr=rs, qq=prev[4],
                               f=norm_phase2: f(st, rr, qq))
                prev = None
            while pending:
                pending.pop(0)()

    nc.compile()
    return nc


def _get_nc():
    if "nc" not in _cache:
        _cache["nc"] = _build()
    return _cache["nc"]


def _tile_w(w):
    # [1024, 128] -> [128, 8, 128] (partition-major tiles)
    return w.reshape(KT_PROJ, 128, HSLICE).transpose(1, 0, 2)


def _in_maps(x, Wq, bq, Wk, bk, Wv, bv, Wo):
    x = np.ascontiguousarray(np.asarray(x, dtype=np.float32))
    xt = np.ascontiguousarray(x.reshape(BS, D).T).astype(np.float16)

    in_maps = []
    for c in range(NCORES):
        sl = slice(c * HSLICE, (c + 1) * HSLICE)
        wq = (np.asarray(Wq, np.float32)[:, sl] / 8.0).astype(np.float16)
        wk = np.asarray(Wk, np.float32)[:, sl].astype(np.float16)
        wv = np.asarray(Wv, np.float32)[:, sl].astype(np.float16)
        w3 = np.ascontiguousarray(
            np.stack([_tile_w(wq), _tile_w(wk), _tile_w(wv)], axis=1)
            .reshape(128, -1))
        b3 = np.stack([np.asarray(bq, np.float32)[sl] / 8.0,
                       np.asarray(bk, np.float32)[sl],
                       np.asarray(bv, np.float32)[sl]], axis=1)
        in_maps.append({
            "xt": xt,
            "w3": w3,
            "b3": np.ascontiguousarray(b3),
            "wo": np.ascontiguousarray(np.asarray(Wo, np.float32)[sl, :]).astype(np.float16),
            "idt": np.eye(128, dtype=np.float16),
        })
    return in_maps


def kernel(x, Wq, bq, Wk, bk, Wv, bv, Wo, bo):
    from concourse.bass_utils import run_bass_kernel_spmd

    nc = _get_nc()
    in_maps = _in_maps(x, Wq, bq, Wk, bk, Wv, bv, Wo)

    res = run_bass_kernel_spmd(nc, in_maps, core_ids=list(range(NCORES)),
                               trace=bool(int(os.environ.get("KTRACE", "0"))))
    _cache["last_result"] = res
    acc = res.results[0]["out"].astype(np.float32)
    for c in range(1, NCORES):
        acc += res.results[c]["out"].astype(np.float32)
    acc += np.asarray(bo, np.float32)[None, :]
    return acc.reshape(B, S, D)
